# revision 20
# baseline (speedup 1.0000x reference)
"""DCRNN encoder (T=24, B=64, N=207, IN=2, H=64, K=2, L=2) on 8 TRN2 cores.

Sharding: data-parallel over batch (8 batches/core), weights + supports
replicated. Each core runs the full 24-step scan for its batch shard; no
collectives. Host packs inputs into kernel-friendly layouts and unpacks
outputs.

Device layouts per core (b = 8 local batches):
  fm (feature-major): SBUF [feat, b*207]  col = b*207 + n
  nm (node-major):    SBUF [node_chunk(128|79), ch, j*8+b]  (j = feature)

Per step, per layer: gates = sigmoid(sum_k A_k cat(x,h) Wk + bg),
cand = tanh(sum_k A_k cat(x, r*h) Wk + bc), h' = u*h + (1-u)*c, with
A = [I, S, M2], M2 = 2 S@S - I (host-precomputed). Diffusion matmuls use
the activation as the stationary operand (lhsT = per-batch-pair nm tile)
against rhs [S^T | M2^T], which yields feature-major outputs directly.
"""

import numpy as np
from contextlib import ExitStack

import concourse.bass as bass
import concourse.bacc as bacc
import concourse.tile as tile
from concourse import mybir
from concourse.bass_utils import run_bass_kernel_spmd

T, B, N, IN, H = 24, 64, 207, 2, 64
NCORES = 8
BSH = B // NCORES          # 8 batches per core
F = BSH * N                # 1656 fm columns
NCH = (128, 79)            # node chunks
NSL = 4                    # gemm column slices (one per batch pair)
SLW = 2 * N                # slice width = 414
f32 = mybir.dt.float32
f32r = mybir.dt.float32r
AF = mybir.ActivationFunctionType


# ---------------------------------------------------------------- program ---

def build_program(t_steps=T):
    nc = bacc.Bacc("TRN2", target_bir_lowering=False, debug=False,
                   num_devices=NCORES)
    d = {}

    def din(name, shape, dt=f32r):
        d[name] = nc.dram_tensor(name, list(shape), dt, kind="ExternalInput").ap()

    def dout(name, shape, dt=f32r):
        d[name] = nc.dram_tensor(name, list(shape), dt, kind="ExternalOutput").ap()

    din("sm2t", (2, 128, SLW))        # [S^T | M2^T] per node chunk, zero-padded
    din("ident", (128, 128))
    din("identhi", (128, 64))
    for nm_, o in (("wg0", 128), ("wc0", 64)):
        din(nm_ + "a", (64, o)); din(nm_ + "bS", (64, o))
        din(nm_ + "bM", (64, o)); din(nm_ + "c", (6, o))
    for nm_, o in (("wg1", 128), ("wc1", 64)):
        din(nm_ + "b0", (128, o))
        for s in ("xS", "hS", "xM", "hM"):
            din(nm_ + s, (64, o))
    din("bg0", (128, 1), f32); din("bc0", (64, 1), f32)
    din("bg1", (128, 1), f32); din("bc1", (64, 1), f32)
    din("x0f6", (t_steps, 6, F))         # [t, (x|Sx|M2x)(c), b*207+n]
    din("hfm0", (128, F))                # [l*64+j, b*207+n]
    din("hnm0", (2, 128, 1024))          # [ch, m, j*8+b], j = l*64+jj
    dout("cur", (t_steps, 2, 128, 512))  # [t, ch, m, jj*8+b] = h1
    dout("hlast", (2, 128, 1024))        # final [ch, m, (l*64+jj)*8+b]

    with tile.TileContext(nc) as tc, ExitStack() as ctx:
        _emit(ctx, tc, nc, d, t_steps)
    nc.compile()
    return nc


def _emit(ctx, tc, nc, d, t_steps):
    cp = ctx.enter_context(tc.tile_pool(name="consts", bufs=1))
    sp = ctx.enter_context(tc.tile_pool(name="work", bufs=1))
    pp = ctx.enter_context(tc.tile_pool(name="ps", bufs=1, space="PSUM"))
    dma = nc.sync.dma_start
    _ev = [0]

    def evac(out, in_):
        _ev[0] ^= 1
        if _ev[0]:
            nc.scalar.copy(out, in_)
        else:
            nc.vector.tensor_copy(out, in_)

    # ---- constants
    smt = cp.tile([128, 2, SLW], f32r, name="smt")
    for ch in range(2):
        dma(smt[:, ch, :], d["sm2t"][ch])
    ident = cp.tile([128, 128], f32r, name="ident")
    dma(ident[:], d["ident"][:])
    identhi = cp.tile([128, 64], f32r, name="identhi")
    dma(identhi[:], d["identhi"][:])
    # L0 weights: A: h-block; bS/bM: Sh/M2h blocks; C: x|Sx|M2x rows
    wg0a = cp.tile([64, 128], f32r, name="wg0a"); dma(wg0a[:], d["wg0a"][:])
    wg0bS = cp.tile([64, 128], f32r, name="wg0bS"); dma(wg0bS[:], d["wg0bS"][:])
    wg0bM = cp.tile([64, 128], f32r, name="wg0bM"); dma(wg0bM[:], d["wg0bM"][:])
    wg0c = cp.tile([6, 128], f32r, name="wg0c"); dma(wg0c[:], d["wg0c"][:])
    wc0a = cp.tile([64, 64], f32r, name="wc0a"); dma(wc0a[:], d["wc0a"][:])
    wc0bS = cp.tile([64, 64], f32r, name="wc0bS"); dma(wc0bS[:], d["wc0bS"][:])
    wc0bM = cp.tile([64, 64], f32r, name="wc0bM"); dma(wc0bM[:], d["wc0bM"][:])
    wc0c = cp.tile([6, 64], f32r, name="wc0c"); dma(wc0c[:], d["wc0c"][:])
    # L1 weights: b0: block0 [128]; xS/hS/xM/hM: 64-row splits of blocks 1/2
    wg1b0 = cp.tile([128, 128], f32r, name="wg1b0"); dma(wg1b0[:], d["wg1b0"][:])
    wg1xS = cp.tile([64, 128], f32r, name="wg1xS"); dma(wg1xS[:], d["wg1xS"][:])
    wg1hS = cp.tile([64, 128], f32r, name="wg1hS"); dma(wg1hS[:], d["wg1hS"][:])
    wg1xM = cp.tile([64, 128], f32r, name="wg1xM"); dma(wg1xM[:], d["wg1xM"][:])
    wg1hM = cp.tile([64, 128], f32r, name="wg1hM"); dma(wg1hM[:], d["wg1hM"][:])
    wc1b0 = cp.tile([128, 64], f32r, name="wc1b0"); dma(wc1b0[:], d["wc1b0"][:])
    wc1xS = cp.tile([64, 64], f32r, name="wc1xS"); dma(wc1xS[:], d["wc1xS"][:])
    wc1hS = cp.tile([64, 64], f32r, name="wc1hS"); dma(wc1hS[:], d["wc1hS"][:])
    wc1xM = cp.tile([64, 64], f32r, name="wc1xM"); dma(wc1xM[:], d["wc1xM"][:])
    wc1hM = cp.tile([64, 64], f32r, name="wc1hM"); dma(wc1hM[:], d["wc1hM"][:])
    bg0 = cp.tile([128, 1], f32, name="bg0"); dma(bg0[:], d["bg0"][:])
    bc0 = cp.tile([64, 1], f32, name="bc0"); dma(bc0[:], d["bc0"][:])
    bg1 = cp.tile([128, 1], f32, name="bg1"); dma(bg1[:], d["bg1"][:])
    bc1 = cp.tile([64, 1], f32, name="bc1"); dma(bc1[:], d["bc1"][:])

    def wtile(name, shape, bufs, t, dt=None):
        return sp.tile(shape, dt or f32r, tag=name, bufs=bufs, name=f"{name}_{t}")

    def gps(name, t):
        return pp.tile([128, 1024], f32, tag="gps", bufs=2, name=f"{name}_{t}")

    def trps(name, t):
        return pp.tile([128, 512], f32r, tag="gps", bufs=2, name=f"{name}_{t}")

    def sl2(q):
        return slice(q * 2 * SLW, (q + 1) * 2 * SLW)

    def ps_view(ps, rows):
        return ps[rows, :].rearrange("p (s w) -> p s w", w=512)[:, :, 0:SLW]

    def fm_view(tl, rows, q):
        return tl[rows, sl2(q)].rearrange("p (s w) -> p s w", w=SLW)

    def pair_diff(nm_tile, tag, t):
        """one [128, 2048] psum; pair p at cols 512p: [S | M2] blocks."""
        ps = pp.tile([128, 2048], f32, tag="dps", bufs=1, name=f"{tag}_{t}")
        for p in range(NSL):
            for ch in range(2):
                nc.tensor.matmul(
                    ps[:, 512 * p:512 * p + SLW],
                    nm_tile[0:NCH[ch], ch, 128 * p:128 * (p + 1)],
                    smt[0:NCH[ch], ch, :], start=(ch == 0), stop=(ch == 1))
        return ps

    def diff_evac(ps, dstS, dstM):
        """4 strided copies [64, 4, 207]: (pair-member i) x (S | M2)."""
        for i in range(2):
            for role, dst in ((0, dstS), (1, dstM)):
                src = ps[64 * i:64 * i + 64, :].rearrange(
                    "p (s w) -> p s w", w=512)[:, :, role * N:role * N + N]
                dv = dst[0:64, :].rearrange(
                    "p (b n) -> p b n", n=N)[:, i::2, :]
                evac(dv, src)

    def tr_group(src, src_row, dst_nm, idt, tag, t):
        for ch in range(2):
            ps = trps(f"{tag}{ch}", t)
            for b in range(BSH):
                cols = slice(b * N + 128 * ch, b * N + 128 * ch + NCH[ch])
                nc.tensor.matmul(ps[0:NCH[ch], b * 64:(b + 1) * 64],
                                 src[src_row:src_row + 64, cols], idt,
                                 is_transpose=True, start=(b == 0), stop=(b == 7))
            evac(dst_nm[0:NCH[ch], ch, :], ps[0:NCH[ch], :])

    # ---- initial state
    # hx_fm(t) = [h0(t+1) | h1(t)]; hx_fm_prev rows 0:64 give h0(t) to L0.
    # sx0/m2x0 = [S h0(t)], [M2 h0(t)] from G1 of step t-1 (prologue for t=0).
    hx_fm_p = wtile("hx_fm", [128, F], 2, "init")
    dma(hx_fm_p[0:64, :], d["hfm0"][0:64, :])
    hx_fm = wtile("hx_fm", [128, F], 2, 0)
    dma(hx_fm[64:128, :], d["hfm0"][64:128, :])
    h1fm = wtile("h1fm", [64, F], 2, 0, f32)
    dma(h1fm[:], d["hfm0"][64:128, :].bitcast(f32))
    h0nm_p = wtile("h0nm", [128, 2, 512], 2, "init")
    h1nm = wtile("h1nm", [128, 2, 512], 2, 0)
    for ch in range(2):
        dma(h0nm_p[0:NCH[ch], ch, :], d["hnm0"][ch, 0:NCH[ch], 0:512])
        dma(h1nm[0:NCH[ch], ch, :], d["hnm0"][ch, 0:NCH[ch], 512:1024])
    sx0 = wtile("sx0", [64, F], 2, 0)
    m2x0 = wtile("m2x0", [64, F], 2, 0)
    diff_evac(pair_diff(h0nm_p, "d0p", "init"), sx0, m2x0)

    for t in range(t_steps):
        hx_fm_n = wtile("hx_fm", [128, F], 2, t + 1)
        h1fm_n = wtile("h1fm", [64, F], 2, t + 1, f32)
        sx0_n = wtile("sx0", [64, F], 2, t + 1)
        m2x0_n = wtile("m2x0", [64, F], 2, t + 1)
        c1ch0 = wtile("c1ch0", [128, F], 2, t)
        h1nm_n = wtile("h1nm", [128, 2, 512], 2, t + 1)
        x6 = wtile("x6", [6, F], 2, t)
        dma(x6[:], d["x0f6"][t])

        # ---------- L0 gates GEMM + sigmoid -> r0/u0
        r0 = wtile("r0", [64, F], 1, t, f32)
        u0 = wtile("u0", [64, F], 1, t, f32)
        for q in range(2):
            ps = gps(f"g0{q}", t)
            for pl in range(2):
                sl = slice((2 * q + pl) * SLW, (2 * q + pl + 1) * SLW)
                po = slice(512 * pl, 512 * pl + SLW)
                nc.tensor.matmul(ps[:, po], wg0a[:], hx_fm_p[0:64, sl],
                                 start=True, stop=False)
                nc.tensor.matmul(ps[:, po], wg0bS[:], sx0[:, sl],
                                 start=False, stop=False)
                nc.tensor.matmul(ps[:, po], wg0bM[:], m2x0[:, sl],
                                 start=False, stop=False)
                nc.tensor.matmul(ps[:, po], wg0c[:], x6[:, sl],
                                 start=False, stop=True)
            nc.scalar.activation(fm_view(r0, slice(0, 64), q),
                                 ps_view(ps, slice(0, 64)),
                                 AF.Sigmoid, bias=bg0[0:64, 0:1])
            nc.scalar.activation(fm_view(u0, slice(0, 64), q),
                                 ps_view(ps, slice(64, 128)),
                                 AF.Sigmoid, bias=bg0[64:128, 0:1])

        # ---------- L0 cand: z0, transpose, diffuse
        z0t = wtile("z0t", [64, F], 1, t)
        for q in range(2):
            nc.vector.tensor_mul(z0t[:, sl2(q)], r0[:, sl2(q)],
                                 hx_fm_p[0:64, sl2(q)].bitcast(f32))
        z0nm = wtile("z0nm", [128, 2, 512], 1, t)
        tr_group(z0t, 0, z0nm, ident[0:64, 0:64], "tz0", t)
        sz0 = wtile("sz0", [64, F], 1, t)
        m2z0 = wtile("m2z0", [64, F], 1, t)
        diff_evac(pair_diff(z0nm, "dz0", t), sz0, m2z0)

        # ---------- L0 cand GEMM + tanh + h0'
        c0 = sp.tile([64, F], f32, tag="cc", bufs=1, name=f"c0_{t}")
        tmp0 = sp.tile([64, F], f32, tag="tmp", bufs=1, name=f"tmp0_{t}")
        for q in range(2):
            ps = gps(f"c0{q}", t)
            for pl in range(2):
                sl = slice((2 * q + pl) * SLW, (2 * q + pl + 1) * SLW)
                po = slice(512 * pl, 512 * pl + SLW)
                nc.tensor.matmul(ps[0:64, po], wc0a[:], z0t[:, sl],
                                 start=True, stop=False)
                nc.tensor.matmul(ps[0:64, po], wc0bS[:], sz0[:, sl],
                                 start=False, stop=False)
                nc.tensor.matmul(ps[0:64, po], wc0bM[:], m2z0[:, sl],
                                 start=False, stop=False)
                nc.tensor.matmul(ps[0:64, po], wc0c[:], x6[:, sl],
                                 start=False, stop=True)
            cv = fm_view(c0, slice(0, 64), q)
            nc.scalar.activation(cv, ps_view(ps, slice(0, 64)), AF.Tanh,
                                 bias=bc0[:, 0:1])
            tv = fm_view(tmp0, slice(0, 64), q)
            nc.vector.tensor_sub(tv, fm_view(hx_fm_p, slice(0, 64), q).bitcast(f32), cv)
            nc.vector.tensor_mul(tv, fm_view(u0, slice(0, 64), q), tv)
            nc.vector.tensor_add(fm_view(hx_fm, slice(0, 64), q), tv, cv)
        nc.gpsimd.tensor_copy(c1ch0[0:64, :], hx_fm[0:64, :])

        # ---------- h0(t+1) -> nm; G2 (h1) then G1 (h0') diffusion
        h0nm = wtile("h0nm", [128, 2, 512], 2, t)
        tr_group(hx_fm, 0, h0nm, ident[0:64, 0:64], "th0", t)
        sh1 = wtile("sh1", [64, F], 1, t)
        m2h1 = wtile("m2h1", [64, F], 1, t)
        diff_evac(pair_diff(h1nm, "d1h", t), sh1, m2h1)
        diff_evac(pair_diff(h0nm, "d1x", t), sx0_n, m2x0_n)

        # ---------- L1 gates GEMM + sigmoid
        r1 = wtile("r1", [64, F], 1, t, f32)
        u1 = wtile("u1", [64, F], 1, t, f32)
        for q in range(2):
            ps = gps(f"g1{q}", t)
            for pl in range(2):
                sl = slice((2 * q + pl) * SLW, (2 * q + pl + 1) * SLW)
                po = slice(512 * pl, 512 * pl + SLW)
                nc.tensor.matmul(ps[:, po], wg1b0[:], hx_fm[:, sl],
                                 start=True, stop=False)
                nc.tensor.matmul(ps[:, po], wg1xS[:], sx0_n[:, sl],
                                 start=False, stop=False)
                nc.tensor.matmul(ps[:, po], wg1hS[:], sh1[:, sl],
                                 start=False, stop=False)
                nc.tensor.matmul(ps[:, po], wg1xM[:], m2x0_n[:, sl],
                                 start=False, stop=False)
                nc.tensor.matmul(ps[:, po], wg1hM[:], m2h1[:, sl],
                                 start=False, stop=True)
            nc.scalar.activation(fm_view(r1, slice(0, 64), q),
                                 ps_view(ps, slice(0, 64)),
                                 AF.Sigmoid, bias=bg1[0:64, 0:1])
            nc.scalar.activation(fm_view(u1, slice(0, 64), q),
                                 ps_view(ps, slice(64, 128)),
                                 AF.Sigmoid, bias=bg1[64:128, 0:1])

        # ---------- L1 cand: z1, transpose, diffuse
        for q in range(2):
            nc.vector.tensor_mul(c1ch0[64:128, sl2(q)],
                                 r1[:, sl2(q)], h1fm[:, sl2(q)])
        z1nm = wtile("z1nm", [128, 2, 512], 1, t)
        tr_group(c1ch0, 64, z1nm, identhi[64:128, :], "tz1", t)
        sz1 = wtile("sz1", [64, F], 1, t)
        m2z1 = wtile("m2z1", [64, F], 1, t)
        diff_evac(pair_diff(z1nm, "dz1", t), sz1, m2z1)

        # ---------- L1 cand GEMM + tanh + h1'
        c1 = sp.tile([64, F], f32, tag="cc", bufs=1, name=f"c1_{t}")
        tmp1 = sp.tile([64, F], f32, tag="tmp", bufs=1, name=f"tmp1_{t}")
        for q in range(2):
            ps = gps(f"cc1{q}", t)
            for pl in range(2):
                sl = slice((2 * q + pl) * SLW, (2 * q + pl + 1) * SLW)
                po = slice(512 * pl, 512 * pl + SLW)
                nc.tensor.matmul(ps[0:64, po], wc1b0[:], c1ch0[:, sl],
                                 start=True, stop=False)
                nc.tensor.matmul(ps[0:64, po], wc1xS[:], sx0_n[:, sl],
                                 start=False, stop=False)
                nc.tensor.matmul(ps[0:64, po], wc1hS[:], sz1[:, sl],
                                 start=False, stop=False)
                nc.tensor.matmul(ps[0:64, po], wc1xM[:], m2x0_n[:, sl],
                                 start=False, stop=False)
                nc.tensor.matmul(ps[0:64, po], wc1hM[:], m2z1[:, sl],
                                 start=False, stop=True)
            cv = fm_view(c1, slice(0, 64), q)
            nc.scalar.activation(cv, ps_view(ps, slice(0, 64)), AF.Tanh,
                                 bias=bc1[:, 0:1])
            tv = fm_view(tmp1, slice(0, 64), q)
            nc.vector.tensor_sub(tv, fm_view(h1fm, slice(0, 64), q), cv)
            nc.vector.tensor_mul(tv, fm_view(u1, slice(0, 64), q), tv)
            nc.vector.tensor_add(fm_view(hx_fm_n, slice(64, 128), q), tv, cv)
        nc.gpsimd.tensor_copy(h1fm_n[:], hx_fm_n[64:128, :].bitcast(f32))

        # ---------- h1(t+1) -> nm, outputs
        tr_group(hx_fm_n, 64, h1nm_n, identhi[64:128, :], "th1", t)
        dma(d["cur"][t, 0], h1nm_n[:, 0, :])
        dma(d["cur"][t, 1, 0:79], h1nm_n[0:79, 1, :])
        if t == t_steps - 1:
            dma(d["hlast"][0, :, 0:512], h0nm[:, 0, :])
            dma(d["hlast"][1, 0:79, 0:512], h0nm[0:79, 1, :])
            dma(d["hlast"][0, :, 512:1024], h1nm_n[:, 0, :])
            dma(d["hlast"][1, 0:79, 512:1024], h1nm_n[0:79, 1, :])

        hx_fm_p, hx_fm = hx_fm, hx_fm_n
        h1fm, sx0, m2x0, h1nm = h1fm_n, sx0_n, m2x0_n, h1nm_n


# ------------------------------------------------------------- host pack ---

def prep_inputs(inputs, initial_hidden_state, supports,
                w_gate0, b_gate0, w_cand0, b_cand0,
                w_gate1, b_gate1, w_cand1, b_cand1, t_steps=T):
    S = np.asarray(supports[0], np.float32)
    M2 = (2.0 * S @ S - np.eye(N, dtype=np.float32)).astype(np.float32)
    sm2t = np.zeros((2, 128, SLW), np.float32)
    for ch in range(2):
        m0, msz = 128 * ch, NCH[ch]
        sm2t[ch, 0:msz, 0:N] = S.T[m0:m0 + msz]
        sm2t[ch, 0:msz, N:2 * N] = M2.T[m0:m0 + msz]

    def l0_rows(pre, w):
        blk = [w[66 * k:66 * (k + 1)] for k in range(3)]
        return {pre + "a": blk[0][2:66], pre + "bS": blk[1][2:66],
                pre + "bM": blk[2][2:66],
                pre + "c": np.concatenate([b[0:2] for b in blk], 0)}

    def l1_rows(pre, w):
        blk = [w[128 * k:128 * (k + 1)] for k in range(3)]
        return {pre + "b0": blk[0], pre + "xS": blk[1][0:64],
                pre + "hS": blk[1][64:128], pre + "xM": blk[2][0:64],
                pre + "hM": blk[2][64:128]}

    wdict = {}
    wdict.update(l0_rows("wg0", np.asarray(w_gate0, np.float32)))
    wdict.update(l0_rows("wc0", np.asarray(w_cand0, np.float32)))
    wdict.update(l1_rows("wg1", np.asarray(w_gate1, np.float32)))
    wdict.update(l1_rows("wc1", np.asarray(w_cand1, np.float32)))
    wdict = {k: np.ascontiguousarray(v) for k, v in wdict.items()}

    x = np.asarray(inputs, np.float32)[:t_steps]          # [t, B, N, IN]
    xf = x.transpose(2, 0, 1, 3).reshape(N, -1)           # [n, t*B*c]
    sx = (S @ xf).reshape(N, t_steps, B, IN).transpose(1, 2, 0, 3)
    m2x = (M2 @ xf).reshape(N, t_steps, B, IN).transpose(1, 2, 0, 3)
    h0 = np.asarray(initial_hidden_state, np.float32)     # [L, B, N*H]

    shared = {
        "sm2t": sm2t, "ident": np.eye(128, dtype=np.float32),
        "identhi": np.vstack([np.zeros((64, 64), np.float32),
                              np.eye(64, dtype=np.float32)]),
        **wdict,
        "bg0": np.asarray(b_gate0, np.float32).reshape(128, 1),
        "bc0": np.asarray(b_cand0, np.float32).reshape(64, 1),
        "bg1": np.asarray(b_gate1, np.float32).reshape(128, 1),
        "bc1": np.asarray(b_cand1, np.float32).reshape(64, 1),
    }

    in_maps = []
    for c in range(NCORES):
        bs = slice(c * BSH, (c + 1) * BSH)
        xc = x[:, bs]                                     # [t, 8, N, 2]
        x0f6 = np.empty((t_steps, 6, F), np.float32)
        x0f6[:, 0:2] = xc.transpose(0, 3, 1, 2).reshape(t_steps, 2, F)
        x0f6[:, 2:4] = sx[:, bs].transpose(0, 3, 1, 2).reshape(t_steps, 2, F)
        x0f6[:, 4:6] = m2x[:, bs].transpose(0, 3, 1, 2).reshape(t_steps, 2, F)
        hc = h0[:, bs].reshape(2, BSH, N, H)              # [l, b, n, j]
        hfm0 = np.ascontiguousarray(
            hc.transpose(0, 3, 1, 2).reshape(128, F))
        hnm0 = np.zeros((2, 128, 1024), np.float32)
        for ch in range(2):
            m0, msz = 128 * ch, NCH[ch]
            # [m, l, b, j] <- [l, b, m, j]
            hnm0[ch, 0:msz] = hc[:, :, m0:m0 + msz, :].transpose(
                2, 0, 1, 3).reshape(msz, 1024)
        in_maps.append(dict(shared, x0f6=x0f6, hfm0=hfm0, hnm0=hnm0))
    return in_maps


def post_outputs(results, t_steps=T):
    cur = np.empty((t_steps, B, N, H), np.float32)
    hid = np.empty((2, B, N, H), np.float32)
    for c, r in enumerate(results):
        bs = slice(c * BSH, (c + 1) * BSH)
        cc = r["cur"].reshape(t_steps, 2 * 128, BSH, H)   # [t, n, b, j]
        cur[:, bs] = cc[:, 0:N].transpose(0, 2, 1, 3)
        hl = r["hlast"].reshape(2 * 128, 2, BSH, H)       # [n, l, b, j]
        hid[:, bs] = hl[0:N].transpose(1, 2, 0, 3)
    return (hid.reshape(2, B, N * H),
            cur.reshape(t_steps, B, N * H))


_CACHE = {}


def kernel(**inputs):
    nc = _CACHE.get("nc")
    if nc is None:
        nc = build_program(T)
        _CACHE["nc"] = nc
    in_maps = prep_inputs(**inputs)
    res = run_bass_kernel_spmd(nc, in_maps, list(range(NCORES)))
    return post_outputs(res.results)
